# revision 1
# baseline (speedup 1.0000x reference)
"""RNN(LSTM)+additive-attention language model on 8 trn2 cores.

Sharding: every core runs the full LSTM (both batches, merged into one set of
[128, 4] state tiles); core c = (b, ib) then does attention + vocab projection
for query rows [ib*128, (ib+1)*128) of batch b. Per-core row selection is done
with indirect-DMA gathers driven by per-core int32 index inputs, so all 8
cores run one identical SPMD program.
"""

import os
import numpy as np
from contextlib import ExitStack

import concourse.bass as bass
import concourse.tile as tile
from concourse import bacc, mybir
from concourse.bass_utils import run_bass_kernel_spmd
from concourse.masks import make_identity

F32 = mybir.dt.float32
I32 = mybir.dt.int32
AF = mybir.ActivationFunctionType
AX = mybir.AxisListType

B, T, E, H, VOCAB = 2, 512, 256, 256, 32000
NCORES = 8
QB = 128          # query rows per core
VB = 500          # vocab cols per projection block
NVB = VOCAB // VB  # 64


def build():
    nc = bacc.Bacc("TRN2", num_devices=NCORES)

    emb_e = nc.declare_dram_parameter("emb", [VOCAB, E], F32, isOutput=False)
    xt_e = nc.declare_dram_parameter("xt", [128, 8], I32, isOutput=False)
    wih_e = nc.declare_dram_parameter("wihT", [E, 4 * H], F32, isOutput=False)
    whh_e = nc.declare_dram_parameter("whhT", [H, 4 * H], F32, isOutput=False)
    bT_e = nc.declare_dram_parameter("biasT", [128, 8], F32, isOutput=False)
    w1_e = nc.declare_dram_parameter("w1T", [H, H], F32, isOutput=False)
    w2_e = nc.declare_dram_parameter("w2T", [H, H], F32, isOutput=False)
    b12_e = nc.declare_dram_parameter("b12", [1, H], F32, isOutput=False)
    vt_e = nc.declare_dram_parameter("vt", [128, 2], F32, isOutput=False)
    wfc_e = nc.declare_dram_parameter("wfcT", [2 * H, VOCAB], F32, isOutput=False)
    qi_e = nc.declare_dram_parameter("qi", [128, 1], I32, isOutput=False)
    ki_e = nc.declare_dram_parameter("ki", [128, 4], I32, isOutput=False)
    mask_e = nc.declare_dram_parameter("mask", [128, T], F32, isOutput=False)
    out_e = nc.declare_dram_parameter("out", [QB, VOCAB], F32, isOutput=True)

    a_dram = nc.dram_tensor("a_scr", [B * T, H], F32)
    b_dram = nc.dram_tensor("b_scr", [B * T, H], F32)
    o_dram = nc.dram_tensor("o_scr", [B * T, H], F32)

    with tile.TileContext(nc) as tc, ExitStack() as ctx:
        cp = ctx.enter_context(tc.tile_pool(name="cp", bufs=1))
        sp = ctx.enter_context(tc.tile_pool(name="sp", bufs=3))
        wp = ctx.enter_context(tc.tile_pool(name="wp", bufs=8))
        pp = ctx.enter_context(tc.tile_pool(name="pp", bufs=2, space="PSUM"))

        # ---- constants / params ----
        ident = cp.tile([128, 128], F32)
        make_identity(nc, ident)
        ones_s = cp.tile([1, 128], F32)
        nc.vector.memset(ones_s, 1.0)

        wih_s = cp.tile([128, 2 * 4 * H], F32)   # col = kc*1024 + g
        whh_s = cp.tile([128, 2 * 4 * H], F32)
        for kc in range(2):
            nc.sync.dma_start(out=wih_s[:, kc * 1024:(kc + 1) * 1024],
                              in_=wih_e[kc * 128:(kc + 1) * 128, :])
            nc.sync.dma_start(out=whh_s[:, kc * 1024:(kc + 1) * 1024],
                              in_=whh_e[kc * 128:(kc + 1) * 128, :])
        biasT_s = cp.tile([128, 8], F32)
        nc.sync.dma_start(out=biasT_s, in_=bT_e[:])
        w1_s = cp.tile([128, 2 * H], F32)        # col = hcin*256 + hout
        w2_s = cp.tile([128, 2 * H], F32)
        for kc in range(2):
            nc.sync.dma_start(out=w1_s[:, kc * H:(kc + 1) * H],
                              in_=w1_e[kc * 128:(kc + 1) * 128, :])
            nc.sync.dma_start(out=w2_s[:, kc * H:(kc + 1) * H],
                              in_=w2_e[kc * 128:(kc + 1) * 128, :])
        b12_s = cp.tile([1, H], F32)
        nc.sync.dma_start(out=b12_s, in_=b12_e[:])
        vt_s = cp.tile([128, 2], F32)
        nc.sync.dma_start(out=vt_s, in_=vt_e[:])
        xt_s = cp.tile([128, 8], I32)
        nc.sync.dma_start(out=xt_s, in_=xt_e[:])
        qi_s = cp.tile([128, 1], I32)
        nc.sync.dma_start(out=qi_s, in_=qi_e[:])
        ki_s = cp.tile([128, 4], I32)
        nc.sync.dma_start(out=ki_s, in_=ki_e[:])
        mask_s = cp.tile([128, T], F32)
        nc.sync.dma_start(out=mask_s, in_=mask_e[:])

        # ---- embedding gather + transpose -> xeT[b] [128, 2ec*512] ----
        xeT = [cp.tile([128, 2 * T], F32, name=f"xeT{b}") for b in range(B)]
        for b in range(B):
            for tch in range(4):
                xe_rows = sp.tile([128, E], F32)
                nc.gpsimd.indirect_dma_start(
                    out=xe_rows, out_offset=None, in_=emb_e[:],
                    in_offset=bass.IndirectOffsetOnAxis(
                        ap=xt_s[:, b * 4 + tch:b * 4 + tch + 1], axis=0))
                for ec in range(2):
                    trp = pp.tile([128, T], F32, name="big", bufs=3)[:, 0:128]
                    nc.tensor.transpose(trp, xe_rows[:, ec * 128:(ec + 1) * 128], ident)
                    nc.scalar.activation(
                        xeT[b][:, ec * T + tch * 128: ec * T + (tch + 1) * 128],
                        trp, AF.Copy)

        # ---- gx precompute: gxT [128, T*16], col = t*16 + gc*2 + b ----
        gxT = cp.tile([128, T * 16], F32)
        for b in range(B):
            for gc in range(8):
                gx_ps = pp.tile([128, T], F32, name="big", bufs=3)
                for ec in range(2):
                    nc.tensor.matmul(
                        gx_ps,
                        wih_s[:, ec * 1024 + gc * 128: ec * 1024 + (gc + 1) * 128],
                        xeT[b][:, ec * T:(ec + 1) * T],
                        start=(ec == 0), stop=(ec == 1))
                off = gc * 2 + b
                nc.vector.tensor_scalar(
                    out=gxT[:, off: off + 16 * (T - 1) + 1: 16], in0=gx_ps,
                    scalar1=biasT_s[:, gc:gc + 1], scalar2=None,
                    op0=mybir.AluOpType.add)

        # ---- LSTM; state [128, 4] col = kc*2 + b ----
        outT = [cp.tile([128, 2 * T], F32, name=f"outT{b}") for b in range(B)]
        hT = cp.tile([128, 4], F32)
        cT = cp.tile([128, 4], F32)
        nc.vector.memset(hT, 0.0)
        nc.vector.memset(cT, 0.0)
        for t in range(T):
            gps = pp.tile([128, 16], F32, name="gps")
            for gc in range(8):
                for kc in range(2):
                    nc.tensor.matmul(
                        gps[:, gc * 2: gc * 2 + 2],
                        whh_s[:, kc * 1024 + gc * 128: kc * 1024 + (gc + 1) * 128],
                        hT[:, kc * 2: kc * 2 + 2],
                        start=(kc == 0), stop=(kc == 1))
            g_sb = sp.tile([128, 16], F32, name="g_sb")
            nc.vector.tensor_add(out=g_sb, in0=gps, in1=gxT[:, t * 16:(t + 1) * 16])
            act = sp.tile([128, 16], F32, name="act", bufs=4)
            nc.scalar.activation(act[:, 0:12], g_sb[:, 0:12], AF.Sigmoid)
            nc.scalar.activation(act[:, 12:16], g_sb[:, 12:16], AF.Tanh)
            tmp = sp.tile([128, 4], F32, name="tmp")
            nc.vector.tensor_mul(out=tmp, in0=act[:, 0:4], in1=act[:, 12:16])
            cT2 = sp.tile([128, 4], F32, name="cT2", bufs=4)
            nc.vector.tensor_mul(out=cT2, in0=act[:, 4:8], in1=cT)
            nc.vector.tensor_add(out=cT2, in0=cT2, in1=tmp)
            thc = sp.tile([128, 4], F32, name="thc")
            nc.scalar.activation(thc, cT2, AF.Tanh)
            hT2 = sp.tile([128, 4], F32, name="hT2", bufs=4)
            nc.vector.tensor_mul(out=hT2, in0=act[:, 8:12], in1=thc)
            for b in range(B):
                nc.vector.tensor_copy(out=outT[b][:, t: t + 513: 512],
                                      in_=hT2[:, b: b + 3: 2])
            hT, cT = hT2, cT2

        # ---- attention query/key features a,b (t-major) -> DRAM ----
        for b in range(B):
            for tch in range(4):
                for which, (w_s, dram, with_bias) in enumerate(
                        [(w1_s, a_dram, True), (w2_s, b_dram, False)]):
                    f_t = pp.tile([128, T], F32, name="big", bufs=3)
                    f_ps = f_t[:, 0:H]
                    for hc in range(2):
                        nc.tensor.matmul(
                            f_ps,
                            outT[b][:, hc * T + tch * 128: hc * T + (tch + 1) * 128],
                            w_s[:, hc * H:(hc + 1) * H],
                            start=(hc == 0),
                            stop=(False if with_bias else hc == 1))
                    if with_bias:
                        nc.tensor.matmul(f_ps, ones_s, b12_s, start=False, stop=True)
                    f_sb = sp.tile([128, H], F32, name="f_sb", bufs=4)
                    nc.vector.tensor_copy(out=f_sb, in_=f_ps)
                    nc.sync.dma_start(
                        out=dram[b * T + tch * 128: b * T + (tch + 1) * 128, :],
                        in_=f_sb)
                # outputs rows -> DRAM
                o_sb = sp.tile([128, H], F32, name="o_sb", bufs=4)
                for hc in range(2):
                    trp = pp.tile([128, T], F32, name="big", bufs=3)[:, 0:128]
                    nc.tensor.transpose(
                        trp, outT[b][:, hc * T + tch * 128: hc * T + (tch + 1) * 128],
                        ident)
                    nc.scalar.activation(o_sb[:, hc * 128:(hc + 1) * 128], trp, AF.Copy)
                nc.sync.dma_start(
                    out=o_dram[b * T + tch * 128: b * T + (tch + 1) * 128, :],
                    in_=o_sb)

        # ---- gathers for this core's (b, iblock) ----
        aq_rows = cp.tile([128, H], F32)
        nc.gpsimd.indirect_dma_start(
            out=aq_rows, out_offset=None, in_=a_dram[:],
            in_offset=bass.IndirectOffsetOnAxis(ap=qi_s[:, 0:1], axis=0))
        oq_rows = cp.tile([128, H], F32)
        nc.gpsimd.indirect_dma_start(
            out=oq_rows, out_offset=None, in_=o_dram[:],
            in_offset=bass.IndirectOffsetOnAxis(ap=qi_s[:, 0:1], axis=0))
        aq_s = cp.tile([128, H], F32)    # col = hc*128 + q
        oqT_s = cp.tile([128, H], F32)
        for hc in range(2):
            trp = pp.tile([128, T], F32, name="big", bufs=3)[:, 0:128]
            nc.tensor.transpose(trp, aq_rows[:, hc * 128:(hc + 1) * 128], ident)
            nc.scalar.activation(aq_s[:, hc * 128:(hc + 1) * 128], trp, AF.Copy)
            trp2 = pp.tile([128, T], F32, name="big", bufs=3)[:, 0:128]
            nc.tensor.transpose(trp2, oq_rows[:, hc * 128:(hc + 1) * 128], ident)
            nc.scalar.activation(oqT_s[:, hc * 128:(hc + 1) * 128], trp2, AF.Copy)
        bT_s = cp.tile([128, 2 * T], F32)  # col = hc*512 + j
        our = [cp.tile([128, H], F32, name=f"our{jc}") for jc in range(4)]
        for jc in range(4):
            b_rows = sp.tile([128, H], F32, name="b_rows", bufs=4)
            nc.gpsimd.indirect_dma_start(
                out=b_rows, out_offset=None, in_=b_dram[:],
                in_offset=bass.IndirectOffsetOnAxis(ap=ki_s[:, jc:jc + 1], axis=0))
            nc.gpsimd.indirect_dma_start(
                out=our[jc], out_offset=None, in_=o_dram[:],
                in_offset=bass.IndirectOffsetOnAxis(ap=ki_s[:, jc:jc + 1], axis=0))
            for hc in range(2):
                trp = pp.tile([128, T], F32, name="big", bufs=3)[:, 0:128]
                nc.tensor.transpose(trp, b_rows[:, hc * 128:(hc + 1) * 128], ident)
                nc.scalar.activation(
                    bT_s[:, hc * T + jc * 128: hc * T + (jc + 1) * 128], trp, AF.Copy)

        # ---- scores + softmax ----
        sm_s = cp.tile([128, T], F32)
        for q in range(QB):
            sc1 = pp.tile([1, T], F32, name="sc1", bufs=2)
            for hc in range(2):
                th = sp.tile([128, T], F32, name="th", bufs=4)
                nc.scalar.activation(
                    th, bT_s[:, hc * T:(hc + 1) * T], AF.Tanh,
                    bias=aq_s[:, hc * 128 + q: hc * 128 + q + 1])
                nc.tensor.matmul(sc1, vt_s[:, hc:hc + 1], th,
                                 start=(hc == 0), stop=(hc == 1))
            scq = sp.tile([1, T], F32, name="scq", bufs=4)
            nc.vector.tensor_copy(out=scq, in_=sc1)
            nc.sync.dma_start(out=sm_s[q:q + 1, :], in_=scq)
        nc.vector.tensor_add(out=sm_s, in0=sm_s, in1=mask_s)
        nmx = cp.tile([128, 1], F32)
        nc.vector.reduce_max(nmx, sm_s, axis=AX.X, negate=True)
        ex_s = cp.tile([128, T], F32)
        ssum = cp.tile([128, 1], F32)
        nc.scalar.activation(ex_s, sm_s, AF.Exp, bias=nmx, accum_out=ssum)
        rs = cp.tile([128, 1], F32)
        nc.vector.reciprocal(rs, ssum)
        at_s = cp.tile([128, T], F32)
        nc.vector.tensor_scalar(out=at_s, in0=ex_s, scalar1=rs, scalar2=None,
                                op0=mybir.AluOpType.mult)

        # ---- context: ctxT [h, q] ----
        ctx_ps = pp.tile([128, H], F32, name="ctx_ps", bufs=1)
        atT = [cp.tile([128, 128], F32, name=f"atT{jc}") for jc in range(4)]
        for jc in range(4):
            trp = pp.tile([128, T], F32, name="big", bufs=3)[:, 0:128]
            nc.tensor.transpose(trp, at_s[:, jc * 128:(jc + 1) * 128], ident)
            nc.scalar.activation(atT[jc], trp, AF.Copy)
        for hc in range(2):
            for jc in range(4):
                nc.tensor.matmul(ctx_ps[:, hc * 128:(hc + 1) * 128],
                                 our[jc][:, hc * 128:(hc + 1) * 128], atT[jc],
                                 start=(jc == 0), stop=(jc == 3))
        ctxT_s = cp.tile([128, H], F32)
        nc.vector.tensor_copy(out=ctxT_s, in_=ctx_ps)

        # ---- projection: out[q, vocab] ----
        stats = [oqT_s[:, 0:128], oqT_s[:, 128:256],
                 ctxT_s[:, 0:128], ctxT_s[:, 128:256]]
        for vb in range(NVB):
            lg_t = pp.tile([128, T], F32, name="big", bufs=3)
            lg_ps = lg_t[:, 0:VB]
            for kc in range(4):
                wt = wp.tile([128, VB], F32, name="wt")
                nc.sync.dma_start(
                    out=wt, in_=wfc_e[kc * 128:(kc + 1) * 128, vb * VB:(vb + 1) * VB])
                nc.tensor.matmul(lg_ps, stats[kc], wt,
                                 start=(kc == 0), stop=(kc == 3))
            lg_sb = sp.tile([128, VB], F32, name="lg_sb", bufs=4)
            nc.vector.tensor_copy(out=lg_sb, in_=lg_ps)
            nc.sync.dma_start(out=out_e[:, vb * VB:(vb + 1) * VB], in_=lg_sb)

    nc.finalize()
    return nc


_NC = None


def _get_nc():
    global _NC
    if _NC is None:
        _NC = build()
    return _NC


def _prep(inputs):
    x = np.asarray(inputs["x"])
    perm = np.concatenate([np.arange(0, 512), np.arange(768, 1024),
                           np.arange(512, 768)])
    wihT = np.ascontiguousarray(np.asarray(inputs["W_ih"])[perm].T)
    whhT = np.ascontiguousarray(np.asarray(inputs["W_hh"])[perm].T)
    bias = (np.asarray(inputs["b_ih"]) + np.asarray(inputs["b_hh"]))[perm]
    biasT = np.ascontiguousarray(bias.reshape(8, 128).T)
    w1T = np.ascontiguousarray(np.asarray(inputs["W1"]).T)
    w2T = np.ascontiguousarray(np.asarray(inputs["W2"]).T)
    b12 = (np.asarray(inputs["b1"]) + np.asarray(inputs["b2"])).reshape(1, H)
    vt = np.ascontiguousarray(np.asarray(inputs["V"])[0].reshape(2, 128).T)
    wfcT = np.ascontiguousarray(np.asarray(inputs["Wfc"]).T)
    xt = np.zeros((128, 8), np.int32)
    for b in range(B):
        for tch in range(4):
            xt[:, b * 4 + tch] = x[b, tch * 128:(tch + 1) * 128]
    common = dict(
        emb=np.ascontiguousarray(np.asarray(inputs["emb"], np.float32)),
        xt=xt, wihT=wihT, whhT=whhT, biasT=biasT, w1T=w1T, w2T=w2T,
        b12=np.ascontiguousarray(b12.astype(np.float32)), vt=vt,
        wfcT=wfcT)
    r = np.arange(128)
    in_maps = []
    for c in range(NCORES):
        b, ib = divmod(c, 4)
        qi = (b * T + ib * 128 + r).astype(np.int32).reshape(128, 1)
        ki = np.stack([(b * T + jc * 128 + r).astype(np.int32)
                       for jc in range(4)], axis=1)
        mask = np.where(np.arange(T)[None, :] <= (ib * 128 + r)[:, None],
                        np.float32(0.0), np.float32(-1e30)).astype(np.float32)
        m = dict(common)
        m.update(qi=qi, ki=np.ascontiguousarray(ki), mask=mask)
        in_maps.append(m)
    return in_maps


LAST = None


def kernel(**inputs):
    global LAST
    nc = _get_nc()
    in_maps = _prep(inputs)
    trace = bool(os.environ.get("KERNEL_TRACE"))
    try:
        br = run_bass_kernel_spmd(nc, in_maps, list(range(NCORES)), trace=trace)
    except Exception:
        if not trace:
            raise
        br = run_bass_kernel_spmd(nc, in_maps, list(range(NCORES)), trace=False)
    LAST = br
    bfc = np.asarray(inputs["bfc"], np.float32)
    logits = np.empty((B, T, VOCAB), np.float32)
    for c in range(NCORES):
        b, ib = divmod(c, 4)
        logits[b, ib * 128:(ib + 1) * 128, :] = br.results[c]["out"]
    logits += bfc[None, None, :]
    return logits

